# revision 4
# baseline (speedup 1.0000x reference)
"""Bahdanau-attention kernel for Trainium2, 8-core data-parallel over batch.

Problem: context = softmax(w2 . tanh(enc @ W1_enc + hid @ W1_hid + b1)) @ enc
  B=32, S=2048, D=1024.  Each of the 8 cores handles 4 batch elements.

Device-side strategy (per core, per batch b):
  - encT [D, S] (host-transposed) tiles feed the big matmul
    h^T[m-chunk] = sum_k W1_enc[k,m]^T @ encT[k]  (PSUM, fp32 accum)
  - tanh+bias via ACT per-partition bias z[m] = (hid @ W1_hid + b1)[m-chunk]
  - e-scores in column form: e[s-col] = sum_m h^T-slice^T @ w2[m]
  - p = exp(e) (no max subtraction: |e| <= sum|w2| ~ 26, safe in fp32)
  - ctx += p_col^T @ enc_natural_tile ; Z += p_col^T @ ones  (PE)
  - context[b] = ctx / Z

Heavy matmuls run as float32r (via AP bitcast): 1 row/cycle on the PE at
moving-N >= 256 (4x faster than float32), measured ~1.3e-4 matmul rel err.
float32r ISA restrictions: moving free count and PSUM dst count must be even,
dst 8B-aligned at partition 0 — hence the 2-column padding of w2/ones and
paired e-score columns.
"""

import numpy as np
from contextlib import ExitStack

import concourse.bacc as bacc
import concourse.tile as tile
from concourse import mybir
from concourse.bass_utils import run_bass_kernel_spmd

AFT = mybir.ActivationFunctionType
F32 = mybir.dt.float32

B, S, D = 32, 2048, 1024
NCORES = 8
BL = B // NCORES          # 4 batch elements per core
P = 128
KC = D // P               # 8 contraction / output chunks
S_SUB = 512               # seq chunk processed per inner iteration
NSS = S // S_SUB          # 4
J = S_SUB // P            # 4 p-columns per seq chunk

# dtype used on the PE for the heavy matmuls (via bitcast of f32 tiles)
DT = mybir.dt.float32r


def _body(ctx, tc, encT, encN, hidT, w1e, w1h, b1, w2, ones, out):
    nc = tc.nc
    const = ctx.enter_context(tc.tile_pool(name="const", bufs=1))
    wpool = ctx.enter_context(tc.tile_pool(name="wpool", bufs=1))
    epool = ctx.enter_context(tc.tile_pool(name="epool", bufs=2 * KC))
    npool = ctx.enter_context(tc.tile_pool(name="npool", bufs=8))
    hpool = ctx.enter_context(tc.tile_pool(name="hpool", bufs=3))
    spool = ctx.enter_context(tc.tile_pool(name="spool", bufs=2))
    # PSUM budget (8 banks): hp/zp 3 + e_ps 2 + ctx_ps 2 + zsum_ps 1
    ppa = ctx.enter_context(tc.tile_pool(name="ppa", bufs=3, space="PSUM"))
    ppe = ctx.enter_context(tc.tile_pool(name="ppe", bufs=2, space="PSUM"))
    ppc = ctx.enter_context(tc.tile_pool(name="ppc", bufs=1, space="PSUM"))
    ppz = ctx.enter_context(tc.tile_pool(name="ppz", bufs=1, space="PSUM"))

    # --- phase 0: weights + per-batch bias z = hid @ W1_hid + b1 ---
    w1e_t, w1h_t, hid_t, b1_t, w2_t = [], [], [], [], []
    for k in range(KC):
        t = wpool.tile([P, D], DT, name=f"w1e_{k}")
        nc.sync.dma_start(t[:], w1e[k * P:(k + 1) * P, :])
        w1e_t.append(t)
        t = wpool.tile([P, D], F32, name=f"w1h_{k}")
        nc.sync.dma_start(t[:], w1h[k * P:(k + 1) * P, :])
        w1h_t.append(t)
        t = const.tile([P, BL], F32, name=f"hid_{k}")
        nc.sync.dma_start(t[:], hidT[k * P:(k + 1) * P, :])
        hid_t.append(t)
        t = const.tile([P, 1], F32, name=f"b1_{k}")
        nc.sync.dma_start(t[:], b1[k * P:(k + 1) * P, :])
        b1_t.append(t)
        t = const.tile([P, 2], DT, name=f"w2_{k}")
        nc.sync.dma_start(t[:], w2[k * P:(k + 1) * P, :])
        w2_t.append(t)
    ones_t = const.tile([P, 2], DT, name="ones_t")
    nc.sync.dma_start(ones_t[:], ones[:])

    z_sb = []
    for m in range(KC):
        zp = ppa.tile([P, BL], F32, name="zp", tag="ppa_t")
        for k in range(KC):
            nc.tensor.matmul(
                zp[:], lhsT=w1h_t[k][:, m * P:(m + 1) * P], rhs=hid_t[k][:],
                start=(k == 0), stop=(k == KC - 1))
        zt = const.tile([P, BL], F32, name=f"z_{m}")
        nc.vector.tensor_scalar_add(zt[:], zp[:], b1_t[m][:])
        z_sb.append(zt)

    # --- main loop ---
    for b in range(BL):
        ctx_ps = ppc.tile([1, D], F32, name="ctx_ps")
        zsum_ps = ppz.tile([1, 2], F32, name="zsum_ps")
        for ss in range(NSS):
            et = []
            for k in range(KC):
                t = epool.tile([P, S_SUB], DT, name="et", tag="et")
                nc.sync.dma_start(
                    t[:], encT[b, k * P:(k + 1) * P, ss * S_SUB:(ss + 1) * S_SUB])
                et.append(t)
            e_ps = ppe.tile([P, 2 * J], F32, name="e_ps")
            for m in range(KC):
                hp = ppa.tile([P, S_SUB], F32, name="hp", tag="ppa_t")
                for k in range(KC):
                    nc.tensor.matmul(
                        hp[:], lhsT=w1e_t[k][:, m * P:(m + 1) * P],
                        rhs=et[k][:],
                        start=(k == 0), stop=(k == KC - 1))
                h_sb = hpool.tile([P, S_SUB], DT, name="h_sb", tag="h_sb")
                nc.scalar.activation(h_sb[:], hp[:], AFT.Tanh,
                                     bias=z_sb[m][:, b:b + 1])
                for j in range(J):
                    nc.tensor.matmul(
                        e_ps[:, 2 * j:2 * j + 2],
                        lhsT=h_sb[:, j * P:(j + 1) * P],
                        rhs=w2_t[m][:],
                        start=(m == 0 and j == 0),
                        stop=(m == KC - 1 and j == J - 1))
            p_sb = spool.tile([P, 2 * J], DT, name="p_sb", tag="p_sb")
            nc.scalar.activation(p_sb[:], e_ps[:], AFT.Exp)
            for j in range(J):
                nat = npool.tile([P, D], DT, name="nat", tag="nat")
                s0 = ss * S_SUB + j * P
                nc.sync.dma_start(nat[:], encN[b, s0:s0 + P, :])
                first = (ss == 0 and j == 0)
                last = (ss == NSS - 1 and j == J - 1)
                p_col = p_sb[:, 2 * j:2 * j + 1]
                nc.tensor.matmul(ctx_ps[0:1, 0:512], lhsT=p_col,
                                 rhs=nat[:, 0:512],
                                 start=first, stop=last)
                nc.tensor.matmul(ctx_ps[0:1, 512:1024], lhsT=p_col,
                                 rhs=nat[:, 512:1024],
                                 start=first, stop=last)
                nc.tensor.matmul(zsum_ps[0:1, 0:2], lhsT=p_col,
                                 rhs=ones_t[:],
                                 start=first, stop=last)
        zr = spool.tile([1, 1], F32, name="zr", tag="zr")
        nc.vector.reciprocal(zr[:], zsum_ps[0:1, 0:1])
        ctx_sb = spool.tile([1, D], F32, name="ctx_sb", tag="ctx_sb")
        nc.vector.tensor_scalar_mul(ctx_sb[:], ctx_ps[:], zr[:])
        nc.sync.dma_start(out[b:b + 1, :], ctx_sb[:])


def build_program():
    nc = bacc.Bacc("TRN2", target_bir_lowering=False, debug=False,
                   num_devices=NCORES)
    encT = nc.dram_tensor("encT", [BL, D, S], DT, kind="ExternalInput").ap()
    encN = nc.dram_tensor("encN", [BL, S, D], DT, kind="ExternalInput").ap()
    hidT = nc.dram_tensor("hidT", [D, BL], F32, kind="ExternalInput").ap()
    w1e = nc.dram_tensor("w1e", [D, D], DT, kind="ExternalInput").ap()
    w1h = nc.dram_tensor("w1h", [D, D], F32, kind="ExternalInput").ap()
    b1 = nc.dram_tensor("b1", [D, 1], F32, kind="ExternalInput").ap()
    w2 = nc.dram_tensor("w2", [D, 2], DT, kind="ExternalInput").ap()
    ones = nc.dram_tensor("ones", [P, 2], DT, kind="ExternalInput").ap()
    out = nc.dram_tensor("ctx_out", [BL, D], F32, kind="ExternalOutput").ap()
    with tile.TileContext(nc) as tc:
        with ExitStack() as ctx:
            _body(ctx, tc, encT, encN, hidT, w1e, w1h, b1, w2, ones, out)
    nc.compile()
    return nc


def prep_in_maps(inputs):
    enc = np.asarray(inputs["encoder_outputs"], dtype=np.float32)
    hid = np.asarray(inputs["hidden_state"], dtype=np.float32)
    W1 = np.asarray(inputs["W1"], dtype=np.float32)
    b1 = np.asarray(inputs["b1"], dtype=np.float32)
    w2 = np.asarray(inputs["w2"], dtype=np.float32)
    encT = np.ascontiguousarray(enc.transpose(0, 2, 1))
    w1e = np.ascontiguousarray(W1[:D])
    w1h = np.ascontiguousarray(W1[D:])
    b1c = np.ascontiguousarray(b1.reshape(D, 1))
    w2c = np.zeros((D, 2), dtype=np.float32)
    w2c[:, 0] = w2
    ones_np = np.zeros((P, 2), dtype=np.float32)
    ones_np[:, 0] = 1.0
    in_maps = []
    for c in range(NCORES):
        sl = slice(c * BL, (c + 1) * BL)
        in_maps.append({
            "encT": encT[sl],
            "encN": np.ascontiguousarray(enc[sl]),
            "hidT": np.ascontiguousarray(hid[sl].T),
            "w1e": w1e,
            "w1h": w1h,
            "b1": b1c,
            "w2": w2c,
            "ones": ones_np,
        })
    return in_maps


_NC_CACHE = None


def kernel(**inputs):
    global _NC_CACHE
    if _NC_CACHE is None:
        _NC_CACHE = build_program()
    nc = _NC_CACHE
    in_maps = prep_in_maps(inputs)
    res = run_bass_kernel_spmd(nc, in_maps, core_ids=list(range(NCORES)))
    out = np.empty((B, D), dtype=np.float32)
    for c in range(NCORES):
        out[c * BL:(c + 1) * BL] = res.results[c]["ctx_out"]
    return out
